# revision 16
# baseline (speedup 1.0000x reference)
"""Distributed Trainium2 Bass kernel for multi-head causal cross-attention.

Reference computation (B=2, T=2048, E=1024, H=16, d=64):
    q = x @ Wq + bq ; k = y @ Wk + bk ; v = y @ Wv + bv      (per-head reshape)
    att = softmax(q k^T / sqrt(d) + causal_mask)
    out = (att v) @ Wo + bo

Sharding over 8 NeuronCores: data-parallel on batch (2 groups of 4 cores),
tensor-parallel on heads (4 heads = 256 channels per core).  Each core
computes a partial output projection; the 4 partials per batch are summed on
the host (the unshard step), plus the output bias.

Per-core dataflow (all layouts chosen so no on-chip transposes are needed):
  - x^T, y^T loaded straight from DRAM with hardware DMA-transpose (bf16)
  - Q^T,K^T = W^T x^T via bf16 matmuls (W stationary), evicted f32r + bias
  - V in augmented layout [tk, 4*65]: per head 64 value cols + a ones col,
    so the PV matmul (M=65) also produces the softmax denominator row
  - scores computed transposed (S^T: tk on partitions, tq free), causal
    block-skipped; diagonal 128-blocks masked by accumulating a -1e10
    strictly-lower-triangular constant via an extra matmul
  - softmax without max-subtraction (scores ~ N(0,1) after 1/8 scaling):
    exp fused with the 1/8 scale on the scalar engine, f32r output
  - normalization via K=1 broadcast matmuls + fused DVE multiply while
    evicting A^T
  - out partial = A^T chunks (stationary) @ Wo rows (moving), f32r
"""

import sys

if "/opt/trn_rl_repo" not in sys.path:
    sys.path.insert(0, "/opt/trn_rl_repo")

import numpy as np
import ml_dtypes

import concourse.bacc as bacc
from concourse.tile_rust import add_dep_helper
import concourse.mybir as mybir
import concourse.tile as tile
from concourse.bass_utils import run_bass_kernel_spmd

BF16 = mybir.dt.bfloat16
F32 = mybir.dt.float32
F32R = mybir.dt.float32r
AF = mybir.ActivationFunctionType

B, T, E, H = 2, 2048, 1024, 16
D = E // H                  # 64 head dim
N_CORES = 8
CPC = E // 4                # 256 channels per core (4 heads)
NEG = -1.0e10

_CACHE = {}
LAST_RESULT = None


def _build():
    nc = bacc.Bacc("TRN2", target_bir_lowering=False, debug=False, num_devices=N_CORES)

    xt = nc.dram_tensor("xt", [E, T], BF16, kind="ExternalInput").ap()
    yt = nc.dram_tensor("yt", [E, T], BF16, kind="ExternalInput").ap()
    wq = nc.dram_tensor("wq", [E, CPC], BF16, kind="ExternalInput").ap()
    wk = nc.dram_tensor("wk", [E, CPC], BF16, kind="ExternalInput").ap()
    wvaug = nc.dram_tensor("wvaug", [E, 260], BF16, kind="ExternalInput").ap()
    wo = nc.dram_tensor("wo", [CPC, E], BF16, kind="ExternalInput").ap()
    bq = nc.dram_tensor("bq", [CPC, 1], F32, kind="ExternalInput").ap()
    bk = nc.dram_tensor("bk", [CPC, 1], F32, kind="ExternalInput").ap()
    bvaug = nc.dram_tensor("bvaug", [1, 260], BF16, kind="ExternalInput").ap()
    btri = nc.dram_tensor("btri", [128, 128], BF16, kind="ExternalInput").ap()
    ident = nc.dram_tensor("ident", [128, 128], BF16, kind="ExternalInput").ap()
    onesr = nc.dram_tensor("onesr", [1, 128], BF16, kind="ExternalInput").ap()
    out = nc.dram_tensor("out", [T, E], BF16, kind="ExternalOutput").ap()

    with tile.TileContext(nc) as tc:
        with (
            nc.allow_low_precision(reason="f32r intermediates; verified <2e-2 end-to-end"),
            tc.tile_pool(name="big", bufs=1) as big,
            tc.tile_pool(name="pt", bufs=3) as ptp,
            tc.tile_pool(name="small", bufs=2) as sm,
            tc.tile_pool(name="zout", bufs=3) as zp,
        ):
            # ---- constants / weights ----
            ld = []
            btri_t = big.tile([128, 128], BF16, tag="btri", name="btri")
            ld.append(nc.gpsimd.dma_start(btri_t[:], btri[:, :]))
            id_t = big.tile([128, 128], BF16, tag="ident", name="ident")
            ld.append(nc.gpsimd.dma_start(id_t[:], ident[:, :]))
            onesr_t = big.tile([1, 128], BF16, tag="onesr", name="onesr")
            ld.append(nc.gpsimd.dma_start(onesr_t[:], onesr[:, :]))
            bvaug_t = big.tile([1, 260], BF16, tag="bvaug", name="bvaug")
            ld.append(nc.gpsimd.dma_start(bvaug_t[:], bvaug[:, :]))

            bq_t = [big.tile([128, 1], F32, tag=f"bq{p}", name=f"bq{p}") for p in range(2)]
            bk_t = [big.tile([128, 1], F32, tag=f"bk{p}", name=f"bk{p}") for p in range(2)]
            for p in range(2):
                ld.append(nc.gpsimd.dma_start(bq_t[p][:], bq[128 * p : 128 * p + 128, :]))
                ld.append(nc.gpsimd.dma_start(bk_t[p][:], bk[128 * p : 128 * p + 128, :]))

            wk_b = big.tile([128, 8 * CPC], BF16, tag="wk_b", name="wk_b")
            wq_b = big.tile([128, 8 * CPC], BF16, tag="wq_b", name="wq_b")
            wv_b = big.tile([128, 8 * 260], BF16, tag="wv_b", name="wv_b")
            ld.append(nc.gpsimd.dma_start(
                wk_b[:].rearrange("p (j c) -> p j c", j=8),
                wk[:, :].rearrange("(j p) c -> p j c", p=128)))
            ld.append(nc.gpsimd.dma_start(
                wq_b[:].rearrange("p (j c) -> p j c", j=8),
                wq[:, :].rearrange("(j p) c -> p j c", p=128)))
            ld.append(nc.gpsimd.dma_start(
                wv_b[:].rearrange("p (j c) -> p j c", j=8),
                wvaug[:, :].rearrange("(j p) c -> p j c", p=128)))
            wk_t = [wk_b[:, CPC * e : CPC * e + CPC] for e in range(8)]
            wq_t = [wq_b[:, CPC * e : CPC * e + CPC] for e in range(8)]
            wv_t = [wv_b[:, 260 * e : 260 * e + 260] for e in range(8)]
            wo_b = big.tile([128, 2 * E], BF16, tag="wo_b", name="wo_b")
            ld.append(nc.gpsimd.dma_start(
                wo_b[:].rearrange("p (j c) -> p j c", j=2),
                wo[:, :].rearrange("(j p) c -> p j c", p=128)))
            wo_t = [wo_b[:, E * p : E * p + E] for p in range(2)]


            # ---- transposed inputs: host passes x^T/y^T; plain SWDGE loads ----
            yTb = big.tile([128, 8 * T], BF16, tag="yTb", name="yTb")
            xTb = big.tile([128, 8 * T], BF16, tag="xTb", name="xTb")
            for h in range(4):
                js = slice(2 * h, 2 * h + 2)
                nc.gpsimd.dma_start(
                    yTb[:, 4096 * h : 4096 * h + 4096].rearrange("p (j f) -> p j f", j=2),
                    yt[:, :].rearrange("(j p) f -> p j f", p=128)[:, js],
                )
            for h in range(4):
                js = slice(2 * h, 2 * h + 2)
                nc.gpsimd.dma_start(
                    xTb[:, 4096 * h : 4096 * h + 4096].rearrange("p (j f) -> p j f", j=2),
                    xt[:, :].rearrange("(j p) f -> p j f", p=128)[:, js],
                )
            yT = [yTb[:, T * e : T * e + T] for e in range(8)]
            xT = [xTb[:, T * e : T * e + T] for e in range(8)]

            KT = [big.tile([128, T], BF16, tag=f"KT{p}", name=f"KT{p}") for p in range(2)]
            QT = [big.tile([128, T], BF16, tag=f"QT{p}", name=f"QT{p}") for p in range(2)]
            AT = [big.tile([128, T], BF16, tag=f"AT{p}", name=f"AT{p}") for p in range(2)]
            V = [big.tile([128, 260], BF16, tag=f"V{c}", name=f"V{c}") for c in range(16)]

            # ---- fused pipeline over tq-blocks J ----
            # per J: produce the K^T/Q^T column block + V chunks needed, run
            # attention for both pairs (interleaved chunk streams), normalize,
            # then the output projection for these tq columns.  Everything but
            # the scores (s0) and PV accumulators (o..) multiplexes through
            # the two 2-bank s0 PSUM slots.
            with tc.tile_pool(name="psa", bufs=2, space="PSUM") as psa:
                for J in range(4):
                    t4 = J
                    # K^T and Q^T column blocks for this J (both pairs)
                    for p in range(2):
                        for w_t, dst, bias in ((wk_t, KT, bk_t), (wq_t, QT, bq_t)):
                            ps = psa.tile([128, 512], F32, tag="s0", name="qk")
                            for e in range(8):
                                nc.tensor.matmul(
                                    ps[:],
                                    w_t[e][:, 128 * p : 128 * p + 128],
                                    (yT if dst is KT else xT)[e][:, 512 * t4 : 512 * t4 + 512],
                                    start=(e == 0),
                                    stop=(e == 7),
                                )
                            nc.vector.tensor_scalar_add(
                                dst[p][:, 512 * t4 : 512 * t4 + 512], ps[:], bias[p][:, 0:1]
                            )
                    # V chunks 4J..4J+3
                    for c in range(4 * J, 4 * J + 4):
                        psv = psa.tile([128, 260], F32, tag="s0", name="psv")
                        for e in range(8):
                            nc.tensor.matmul(
                                psv[:],
                                yT[e][:, 128 * c : 128 * c + 128],
                                wv_t[e][:],
                                start=(e == 0),
                                stop=False,
                            )
                        nc.tensor.matmul(
                            psv[:], onesr_t[0:1, :], bvaug_t[0:1, :], start=False, stop=True
                        )
                        nc.vector.tensor_copy(V[c][:], psv[:])

                    # attention for this J, both pairs interleaved
                    ov = [
                        [
                            psa.tile([65, 512], F32, tag=f"o{p}{h}", bufs=1, name=f"o{p}{h}")
                            for h in range(2)
                        ]
                        for p in range(2)
                    ]
                    nchunks = 4 * J + 4
                    for i in range(nchunks):
                        r = i - 4 * J
                        full = r < 0
                        lo = 0 if full else 128 * r
                        tqs = slice(512 * J + lo, 512 * J + 512)
                        for p in range(2):
                            o0, o1 = ov[p]
                            s0 = psa.tile([128, 1024], F32, tag="s0", name="s0")
                            nc.tensor.matmul(
                                s0[:, lo:512],
                                KT[p][0:64, 128 * i : 128 * i + 128],
                                QT[p][0:64, tqs],
                                start=True,
                                stop=full,
                            )
                            if not full:
                                nc.tensor.matmul(
                                    s0[:, lo : lo + 128], id_t[:], btri_t[:],
                                    start=False, stop=True,
                                )
                            nc.tensor.matmul(
                                s0[:, 512 + lo : 1024],
                                KT[p][64:128, 128 * i : 128 * i + 128],
                                QT[p][64:128, tqs],
                                start=True,
                                stop=full,
                            )
                            if not full:
                                nc.tensor.matmul(
                                    s0[:, 512 + lo : 512 + lo + 128], id_t[:], btri_t[:],
                                    start=False, stop=True,
                                )
                            pt0 = ptp.tile([128, 1024], BF16, tag="pt0", name="pt0")
                            if full:
                                nc.scalar.activation(pt0[:], s0[:], AF.Exp, scale=0.125)
                            else:
                                s3 = s0[:].rearrange("p (s f) -> p s f", s=2)[:, :, lo:512]
                                p3 = pt0[:].rearrange("p (s f) -> p s f", s=2)[:, :, lo:512]
                                nc.scalar.activation(p3, s3, AF.Exp, scale=0.125)
                            h0 = 65 * (2 * p)
                            h1 = 65 * (2 * p + 1)
                            nc.tensor.matmul(
                                o0[0:65, lo:512],
                                V[i][:, h0 : h0 + 65],
                                pt0[:, lo:512],
                                start=(i == 0),
                                stop=(i == nchunks - 1),
                            )
                            nc.tensor.matmul(
                                o1[0:65, lo:512],
                                V[i][:, h1 : h1 + 65],
                                pt0[:, 512 + lo : 1024],
                                start=(i == 0),
                                stop=(i == nchunks - 1),
                            )
                    # normalize + evict A^T
                    for p in range(2):
                        o0, o1 = ov[p]
                        ro0 = sm.tile([1, 512], F32, tag="ro0", name="ro0")
                        ro1 = sm.tile([1, 512], F32, tag="ro1", name="ro1")
                        nc.vector.tensor_copy(ro0[:], o0[64:65, :])
                        nc.vector.tensor_copy(ro1[:], o1[64:65, :])
                        re0 = sm.tile([1, 512], F32, tag="re0", name="re0")
                        re1 = sm.tile([1, 512], F32, tag="re1", name="re1")
                        # approx recip needs an SBUF source (PSUM source breaks
                        # the bit-trick seed -> 14% error)
                        nc.vector.reciprocal_approx_fast(re0[:], ro0[0:1, :])
                        nc.vector.reciprocal_approx_fast(re1[:], ro1[0:1, :])
                        bs0 = sm.tile([64, 512], F32, tag="bs0", name="bs0")
                        bs1 = sm.tile([64, 512], F32, tag="bs1", name="bs1")
                        nc.gpsimd.partition_broadcast(bs0[:], re0[0:1, :])
                        nc.gpsimd.partition_broadcast(bs1[:], re1[0:1, :])
                        Js = slice(512 * J, 512 * J + 512)
                        nc.vector.tensor_mul(AT[p][0:64, Js], o0[0:64, :], bs0[:])
                        nc.vector.tensor_mul(AT[p][64:128, Js], o1[0:64, :], bs1[:])

                    # output projection for tq-chunks of this J
                    for t in range(4 * J, 4 * J + 4):
                        z = zp.tile([128, E], BF16, tag="z", name="z")
                        for eo in range(2):
                            pz = psa.tile([128, 512], F32, tag="s0", name="pz")
                            nc.tensor.matmul(
                                pz[:],
                                AT[0][:, 128 * t : 128 * t + 128],
                                wo_t[0][:, 512 * eo : 512 * eo + 512],
                                start=True,
                                stop=False,
                            )
                            nc.tensor.matmul(
                                pz[:],
                                AT[1][:, 128 * t : 128 * t + 128],
                                wo_t[1][:, 512 * eo : 512 * eo + 512],
                                start=False,
                                stop=True,
                            )
                            if eo == 0:
                                nc.vector.tensor_copy(z[:, 0:512], pz[:])
                            else:
                                nc.scalar.copy(z[:, 512:1024], pz[:])
                        nc.gpsimd.dma_start(out[128 * t : 128 * t + 128, :], z[:])

    nc.compile()
    return nc


def _get_nc():
    if "nc" not in _CACHE:
        _CACHE["nc"] = _build()
    return _CACHE["nc"]


def _consts():
    if "consts" not in _CACHE:
        bf = ml_dtypes.bfloat16
        btri = np.where(
            np.arange(128)[:, None] > np.arange(128)[None, :], NEG, 0.0
        ).astype(bf)
        ident = np.eye(128, dtype=np.float32).astype(bf)
        onesr = np.ones((1, 128), dtype=np.float32).astype(bf)
        _CACHE["consts"] = (btri, ident, onesr)
    return _CACHE["consts"]


def kernel(
    x, y, mask, Wq, bq, Wk, bk, Wv, bv, Wo, bo, num_heads, trace=False
):
    global LAST_RESULT
    assert int(num_heads) == H
    x = np.asarray(x, dtype=np.float32)
    y = np.asarray(y, dtype=np.float32)
    Wq = np.asarray(Wq, dtype=np.float32)
    Wk = np.asarray(Wk, dtype=np.float32)
    Wv = np.asarray(Wv, dtype=np.float32)
    Wo = np.asarray(Wo, dtype=np.float32)
    bq = np.asarray(bq, dtype=np.float32)
    bk = np.asarray(bk, dtype=np.float32)
    bv = np.asarray(bv, dtype=np.float32)
    bo = np.asarray(bo, dtype=np.float32)

    bf = ml_dtypes.bfloat16
    btri, ident, onesr = _consts()

    xtb = [np.ascontiguousarray(x[b].T).astype(bf) for b in range(B)]
    ytb = [np.ascontiguousarray(y[b].T).astype(bf) for b in range(B)]

    in_maps = []
    for c in range(N_CORES):
        b = c // 4
        g = c % 4
        cols = slice(CPC * g, CPC * g + CPC)
        wv_s = Wv[:, cols]
        bv_s = bv[cols]
        wvaug = np.zeros((E, 260), dtype=np.float32)
        bvaug = np.zeros((1, 260), dtype=np.float32)
        for h in range(4):
            wvaug[:, 65 * h : 65 * h + 64] = wv_s[:, 64 * h : 64 * h + 64]
            bvaug[0, 65 * h : 65 * h + 64] = bv_s[64 * h : 64 * h + 64]
            bvaug[0, 65 * h + 64] = 1.0
        in_maps.append(
            {
                "xt": xtb[b],
                "yt": ytb[b],
                "wq": np.ascontiguousarray(Wq[:, cols]).astype(bf),
                "wk": np.ascontiguousarray(Wk[:, cols]).astype(bf),
                "wvaug": wvaug.astype(bf),
                "wo": np.ascontiguousarray(Wo[cols, :]).astype(bf),
                "bq": np.ascontiguousarray(bq[cols]).reshape(CPC, 1),
                "bk": np.ascontiguousarray(bk[cols]).reshape(CPC, 1),
                "bvaug": bvaug.astype(bf),
                "btri": btri,
                "ident": ident,
                "onesr": onesr,
            }
        )

    nc = _get_nc()
    res = run_bass_kernel_spmd(
        nc, in_maps, core_ids=list(range(N_CORES)), trace=trace
    )
    LAST_RESULT = res

    full = np.zeros((B, T, E), dtype=np.float32)
    for c in range(N_CORES):
        full[c // 4] += res.results[c]["out"].astype(np.float32)
    full += bo
    return full


# revision 18
# speedup vs baseline: 1.0821x; 1.0821x over previous
"""Distributed Trainium2 Bass kernel for multi-head causal cross-attention.

Reference computation (B=2, T=2048, E=1024, H=16, d=64):
    q = x @ Wq + bq ; k = y @ Wk + bk ; v = y @ Wv + bv      (per-head reshape)
    att = softmax(q k^T / sqrt(d) + causal_mask)
    out = (att v) @ Wo + bo

Sharding over 8 NeuronCores: data-parallel on batch (2 groups of 4 cores),
tensor-parallel on heads (4 heads = 256 channels per core).  Each core
computes a partial output projection; the 4 partials per batch are summed on
the host (the unshard step), plus the output bias.

Per-core dataflow (all layouts chosen so no on-chip transposes are needed):
  - x^T, y^T loaded straight from DRAM with hardware DMA-transpose (bf16)
  - Q^T,K^T = W^T x^T via bf16 matmuls (W stationary), evicted f32r + bias
  - V in augmented layout [tk, 4*65]: per head 64 value cols + a ones col,
    so the PV matmul (M=65) also produces the softmax denominator row
  - scores computed transposed (S^T: tk on partitions, tq free), causal
    block-skipped; diagonal 128-blocks masked by accumulating a -1e10
    strictly-lower-triangular constant via an extra matmul
  - softmax without max-subtraction (scores ~ N(0,1) after 1/8 scaling):
    exp fused with the 1/8 scale on the scalar engine, f32r output
  - normalization via K=1 broadcast matmuls + fused DVE multiply while
    evicting A^T
  - out partial = A^T chunks (stationary) @ Wo rows (moving), f32r
"""

import sys

if "/opt/trn_rl_repo" not in sys.path:
    sys.path.insert(0, "/opt/trn_rl_repo")

import numpy as np
import ml_dtypes

import concourse.bacc as bacc
from concourse.tile_rust import add_dep_helper
import concourse.mybir as mybir
import concourse.tile as tile
from concourse.bass_utils import run_bass_kernel_spmd

BF16 = mybir.dt.bfloat16
F32 = mybir.dt.float32
F32R = mybir.dt.float32r
AF = mybir.ActivationFunctionType

B, T, E, H = 2, 2048, 1024, 16
D = E // H                  # 64 head dim
N_CORES = 8
CPC = E // 4                # 256 channels per core (4 heads)
NEG = -1.0e10

_CACHE = {}
LAST_RESULT = None


def _build():
    nc = bacc.Bacc("TRN2", target_bir_lowering=False, debug=False, num_devices=N_CORES)

    xt = nc.dram_tensor("xt", [E, T], BF16, kind="ExternalInput").ap()
    yt = nc.dram_tensor("yt", [E, T], BF16, kind="ExternalInput").ap()
    wq = nc.dram_tensor("wq", [E, CPC], BF16, kind="ExternalInput").ap()
    wk = nc.dram_tensor("wk", [E, CPC], BF16, kind="ExternalInput").ap()
    wvaug = nc.dram_tensor("wvaug", [E, 260], BF16, kind="ExternalInput").ap()
    wo = nc.dram_tensor("wo", [CPC, E], BF16, kind="ExternalInput").ap()
    bq = nc.dram_tensor("bq", [CPC, 1], F32, kind="ExternalInput").ap()
    bk = nc.dram_tensor("bk", [CPC, 1], F32, kind="ExternalInput").ap()
    bvaug = nc.dram_tensor("bvaug", [1, 260], BF16, kind="ExternalInput").ap()
    btri = nc.dram_tensor("btri", [128, 128], BF16, kind="ExternalInput").ap()
    ident = nc.dram_tensor("ident", [128, 128], BF16, kind="ExternalInput").ap()
    onesr = nc.dram_tensor("onesr", [1, 128], BF16, kind="ExternalInput").ap()
    out = nc.dram_tensor("out", [T, E], BF16, kind="ExternalOutput").ap()

    with tile.TileContext(nc) as tc:
        with (
            nc.allow_low_precision(reason="f32r intermediates; verified <2e-2 end-to-end"),
            tc.tile_pool(name="big", bufs=1) as big,
            tc.tile_pool(name="pt", bufs=3) as ptp,
            tc.tile_pool(name="small", bufs=2) as sm,
            tc.tile_pool(name="zout", bufs=3) as zp,
        ):
            # ---- constants / weights ----
            ld = []
            btri_t = big.tile([128, 128], BF16, tag="btri", name="btri")
            ld.append(nc.gpsimd.dma_start(btri_t[:], btri[:, :]))
            id_t = big.tile([128, 128], BF16, tag="ident", name="ident")
            ld.append(nc.gpsimd.dma_start(id_t[:], ident[:, :]))
            onesr_t = big.tile([1, 128], BF16, tag="onesr", name="onesr")
            ld.append(nc.gpsimd.dma_start(onesr_t[:], onesr[:, :]))
            bvaug_t = big.tile([1, 260], BF16, tag="bvaug", name="bvaug")
            ld.append(nc.gpsimd.dma_start(bvaug_t[:], bvaug[:, :]))

            bq_t = [big.tile([128, 1], F32, tag=f"bq{p}", name=f"bq{p}") for p in range(2)]
            bk_t = [big.tile([128, 1], F32, tag=f"bk{p}", name=f"bk{p}") for p in range(2)]
            for p in range(2):
                ld.append(nc.gpsimd.dma_start(bq_t[p][:], bq[128 * p : 128 * p + 128, :]))
                ld.append(nc.gpsimd.dma_start(bk_t[p][:], bk[128 * p : 128 * p + 128, :]))

            wk_b = big.tile([128, 8 * CPC], BF16, tag="wk_b", name="wk_b")
            wq_b = big.tile([128, 8 * CPC], BF16, tag="wq_b", name="wq_b")
            wv_b = big.tile([128, 8 * 260], BF16, tag="wv_b", name="wv_b")
            ld.append(nc.gpsimd.dma_start(
                wk_b[:].rearrange("p (j c) -> p j c", j=8),
                wk[:, :].rearrange("(j p) c -> p j c", p=128)))
            ld.append(nc.gpsimd.dma_start(
                wq_b[:].rearrange("p (j c) -> p j c", j=8),
                wq[:, :].rearrange("(j p) c -> p j c", p=128)))
            ld.append(nc.gpsimd.dma_start(
                wv_b[:].rearrange("p (j c) -> p j c", j=8),
                wvaug[:, :].rearrange("(j p) c -> p j c", p=128)))
            wk_t = [wk_b[:, CPC * e : CPC * e + CPC] for e in range(8)]
            wq_t = [wq_b[:, CPC * e : CPC * e + CPC] for e in range(8)]
            wv_t = [wv_b[:, 260 * e : 260 * e + 260] for e in range(8)]
            wo_b = big.tile([128, 2 * E], BF16, tag="wo_b", name="wo_b")
            ld.append(nc.gpsimd.dma_start(
                wo_b[:].rearrange("p (j c) -> p j c", j=2),
                wo[:, :].rearrange("(j p) c -> p j c", p=128)))
            wo_t = [wo_b[:, E * p : E * p + E] for p in range(2)]


            # ---- transposed inputs: host passes x^T/y^T; plain SWDGE loads ----
            yTb = big.tile([128, 8 * T], BF16, tag="yTb", name="yTb")
            xTb = big.tile([128, 8 * T], BF16, tag="xTb", name="xTb")
            for h in range(4):
                js = slice(2 * h, 2 * h + 2)
                nc.gpsimd.dma_start(
                    yTb[:, 4096 * h : 4096 * h + 4096].rearrange("p (j f) -> p j f", j=2),
                    yt[:, :].rearrange("(j p) f -> p j f", p=128)[:, js],
                )
            for h in range(4):
                js = slice(2 * h, 2 * h + 2)
                nc.gpsimd.dma_start(
                    xTb[:, 4096 * h : 4096 * h + 4096].rearrange("p (j f) -> p j f", j=2),
                    xt[:, :].rearrange("(j p) f -> p j f", p=128)[:, js],
                )
            yT = [yTb[:, T * e : T * e + T] for e in range(8)]
            xT = [xTb[:, T * e : T * e + T] for e in range(8)]

            KT = [big.tile([128, T], BF16, tag=f"KT{p}", name=f"KT{p}") for p in range(2)]
            QT = [big.tile([128, T], BF16, tag=f"QT{p}", name=f"QT{p}") for p in range(2)]
            AT = [big.tile([128, T], BF16, tag=f"AT{p}", name=f"AT{p}") for p in range(2)]
            V = [big.tile([128, 260], BF16, tag=f"V{c}", name=f"V{c}") for c in range(16)]

            # ---- fused pipeline over tq-blocks J ----
            # Per J: attention chunk loop for both pairs, with the non-exp PE
            # work (K^T/Q^T/V production for J+1, output projection for J-1)
            # interleaved between chunks so the scalar engine (exp) never
            # starves.  J=0's own QKV is a prelude; J=3's outproj is a tail.
            with tc.tile_pool(name="psa", bufs=2, space="PSUM") as psa:

                def emit_kq(J, p, which):
                    w_t, dst, bias, src_t = (
                        (wk_t, KT, bk_t, yT) if which == "k" else (wq_t, QT, bq_t, xT)
                    )
                    ps = psa.tile([128, 512], F32, tag="s0", name="qk")
                    for e in range(8):
                        nc.tensor.matmul(
                            ps[:],
                            w_t[e][:, 128 * p : 128 * p + 128],
                            src_t[e][:, 512 * J : 512 * J + 512],
                            start=(e == 0),
                            stop=(e == 7),
                        )
                    nc.vector.tensor_scalar_add(
                        dst[p][:, 512 * J : 512 * J + 512], ps[:], bias[p][:, 0:1]
                    )

                def emit_v(c):
                    psv = psa.tile([128, 260], F32, tag="s0", name="psv")
                    for e in range(8):
                        nc.tensor.matmul(
                            psv[:],
                            yT[e][:, 128 * c : 128 * c + 128],
                            wv_t[e][:],
                            start=(e == 0),
                            stop=False,
                        )
                    nc.tensor.matmul(
                        psv[:], onesr_t[0:1, :], bvaug_t[0:1, :], start=False, stop=True
                    )
                    nc.vector.tensor_copy(V[c][:], psv[:])

                def emit_outproj(t):
                    z = zp.tile([128, E], BF16, tag="z", name="z")
                    for eo in range(2):
                        pz = psa.tile([128, 512], F32, tag="s0", name="pz")
                        nc.tensor.matmul(
                            pz[:],
                            AT[0][:, 128 * t : 128 * t + 128],
                            wo_t[0][:, 512 * eo : 512 * eo + 512],
                            start=True,
                            stop=False,
                        )
                        nc.tensor.matmul(
                            pz[:],
                            AT[1][:, 128 * t : 128 * t + 128],
                            wo_t[1][:, 512 * eo : 512 * eo + 512],
                            start=False,
                            stop=True,
                        )
                        if eo == 0:
                            nc.vector.tensor_copy(z[:, 0:512], pz[:])
                        else:
                            nc.scalar.copy(z[:, 512:1024], pz[:])
                    nc.gpsimd.dma_start(out[128 * t : 128 * t + 128, :], z[:])

                # prelude: QKV for J=0
                for p in range(2):
                    emit_kq(0, p, "k")
                    emit_kq(0, p, "q")
                for c in range(4):
                    emit_v(c)

                for J in range(4):
                    # deferred work: QKV for J+1 and outproj for J-1
                    work = []
                    if J < 3:
                        for p in range(2):
                            work.append(lambda p=p: emit_kq(J + 1, p, "k"))
                            work.append(lambda p=p: emit_kq(J + 1, p, "q"))
                        for c in range(4 * J + 4, 4 * J + 8):
                            work.append(lambda c=c: emit_v(c))
                    if J > 0:
                        for t in range(4 * J - 4, 4 * J):
                            work.append(lambda t=t: emit_outproj(t))

                    ov = [
                        [
                            psa.tile([65, 512], F32, tag=f"a{p}{h}", bufs=1, name=f"a{p}{h}")
                            for h in range(2)
                        ]
                        for p in range(2)
                    ]
                    nchunks = 4 * J + 4
                    for i in range(nchunks):
                        r = i - 4 * J
                        full = r < 0
                        lo = 0 if full else 128 * r
                        tqs = slice(512 * J + lo, 512 * J + 512)
                        for p in range(2):
                            o0, o1 = ov[p]
                            s0 = psa.tile([128, 1024], F32, tag="s0", name="s0")
                            nc.tensor.matmul(
                                s0[:, lo:512],
                                KT[p][0:64, 128 * i : 128 * i + 128],
                                QT[p][0:64, tqs],
                                start=True,
                                stop=full,
                            )
                            if not full:
                                nc.tensor.matmul(
                                    s0[:, lo : lo + 128], id_t[:], btri_t[:],
                                    start=False, stop=True,
                                )
                            nc.tensor.matmul(
                                s0[:, 512 + lo : 1024],
                                KT[p][64:128, 128 * i : 128 * i + 128],
                                QT[p][64:128, tqs],
                                start=True,
                                stop=full,
                            )
                            if not full:
                                nc.tensor.matmul(
                                    s0[:, 512 + lo : 512 + lo + 128], id_t[:], btri_t[:],
                                    start=False, stop=True,
                                )
                            pt0 = ptp.tile([128, 1024], BF16, tag="pt0", name="pt0")
                            if full:
                                nc.scalar.activation(pt0[:], s0[:], AF.Exp, scale=0.125)
                            else:
                                s3 = s0[:].rearrange("p (s f) -> p s f", s=2)[:, :, lo:512]
                                p3 = pt0[:].rearrange("p (s f) -> p s f", s=2)[:, :, lo:512]
                                nc.scalar.activation(p3, s3, AF.Exp, scale=0.125)
                            h0 = 65 * (2 * p)
                            h1 = 65 * (2 * p + 1)
                            nc.tensor.matmul(
                                o0[0:65, lo:512],
                                V[i][:, h0 : h0 + 65],
                                pt0[:, lo:512],
                                start=(i == 0),
                                stop=(i == nchunks - 1),
                            )
                            nc.tensor.matmul(
                                o1[0:65, lo:512],
                                V[i][:, h1 : h1 + 65],
                                pt0[:, 512 + lo : 1024],
                                start=(i == 0),
                                stop=(i == nchunks - 1),
                            )
                        # spread deferred work across the chunk loop
                        nw = len(work)
                        lo_w = nw * i // nchunks
                        hi_w = nw * (i + 1) // nchunks
                        for w in work[lo_w:hi_w]:
                            w()

                    # normalize + evict A^T
                    for p in range(2):
                        o0, o1 = ov[p]
                        ro0 = sm.tile([1, 512], F32, tag="ro0", name="ro0")
                        ro1 = sm.tile([1, 512], F32, tag="ro1", name="ro1")
                        nc.vector.tensor_copy(ro0[:], o0[64:65, :])
                        nc.vector.tensor_copy(ro1[:], o1[64:65, :])
                        re0 = sm.tile([1, 512], F32, tag="re0", name="re0")
                        re1 = sm.tile([1, 512], F32, tag="re1", name="re1")
                        # approx recip needs an SBUF source (PSUM source breaks
                        # the bit-trick seed -> 14% error)
                        nc.vector.reciprocal_approx_fast(re0[:], ro0[0:1, :])
                        nc.vector.reciprocal_approx_fast(re1[:], ro1[0:1, :])
                        bs0 = sm.tile([64, 512], F32, tag="bs0", name="bs0")
                        bs1 = sm.tile([64, 512], F32, tag="bs1", name="bs1")
                        nc.gpsimd.partition_broadcast(bs0[:], re0[0:1, :])
                        nc.gpsimd.partition_broadcast(bs1[:], re1[0:1, :])
                        Js = slice(512 * J, 512 * J + 512)
                        nc.vector.tensor_mul(AT[p][0:64, Js], o0[0:64, :], bs0[:])
                        nc.vector.tensor_mul(AT[p][64:128, Js], o1[0:64, :], bs1[:])

                # tail: outproj for J=3
                for t in range(12, 16):
                    emit_outproj(t)

    nc.compile()
    return nc


def _get_nc():
    if "nc" not in _CACHE:
        _CACHE["nc"] = _build()
    return _CACHE["nc"]


def _consts():
    if "consts" not in _CACHE:
        bf = ml_dtypes.bfloat16
        btri = np.where(
            np.arange(128)[:, None] > np.arange(128)[None, :], NEG, 0.0
        ).astype(bf)
        ident = np.eye(128, dtype=np.float32).astype(bf)
        onesr = np.ones((1, 128), dtype=np.float32).astype(bf)
        _CACHE["consts"] = (btri, ident, onesr)
    return _CACHE["consts"]


def kernel(
    x, y, mask, Wq, bq, Wk, bk, Wv, bv, Wo, bo, num_heads, trace=False
):
    global LAST_RESULT
    assert int(num_heads) == H
    x = np.asarray(x, dtype=np.float32)
    y = np.asarray(y, dtype=np.float32)
    Wq = np.asarray(Wq, dtype=np.float32)
    Wk = np.asarray(Wk, dtype=np.float32)
    Wv = np.asarray(Wv, dtype=np.float32)
    Wo = np.asarray(Wo, dtype=np.float32)
    bq = np.asarray(bq, dtype=np.float32)
    bk = np.asarray(bk, dtype=np.float32)
    bv = np.asarray(bv, dtype=np.float32)
    bo = np.asarray(bo, dtype=np.float32)

    bf = ml_dtypes.bfloat16
    btri, ident, onesr = _consts()

    xtb = [np.ascontiguousarray(x[b].T).astype(bf) for b in range(B)]
    ytb = [np.ascontiguousarray(y[b].T).astype(bf) for b in range(B)]

    in_maps = []
    for c in range(N_CORES):
        b = c // 4
        g = c % 4
        cols = slice(CPC * g, CPC * g + CPC)
        wv_s = Wv[:, cols]
        bv_s = bv[cols]
        wvaug = np.zeros((E, 260), dtype=np.float32)
        bvaug = np.zeros((1, 260), dtype=np.float32)
        for h in range(4):
            wvaug[:, 65 * h : 65 * h + 64] = wv_s[:, 64 * h : 64 * h + 64]
            bvaug[0, 65 * h : 65 * h + 64] = bv_s[64 * h : 64 * h + 64]
            bvaug[0, 65 * h + 64] = 1.0
        in_maps.append(
            {
                "xt": xtb[b],
                "yt": ytb[b],
                "wq": np.ascontiguousarray(Wq[:, cols]).astype(bf),
                "wk": np.ascontiguousarray(Wk[:, cols]).astype(bf),
                "wvaug": wvaug.astype(bf),
                "wo": np.ascontiguousarray(Wo[cols, :]).astype(bf),
                "bq": np.ascontiguousarray(bq[cols]).reshape(CPC, 1),
                "bk": np.ascontiguousarray(bk[cols]).reshape(CPC, 1),
                "bvaug": bvaug.astype(bf),
                "btri": btri,
                "ident": ident,
                "onesr": onesr,
            }
        )

    nc = _get_nc()
    res = run_bass_kernel_spmd(
        nc, in_maps, core_ids=list(range(N_CORES)), trace=trace
    )
    LAST_RESULT = res

    full = np.zeros((B, T, E), dtype=np.float32)
    for c in range(N_CORES):
        full[c // 4] += res.results[c]["out"].astype(np.float32)
    full += bo
    return full
